# revision 47
# baseline (speedup 1.0000x reference)
"""Masked multi-head attention (B=2, H=16, S=2048, D=64) on 8 TRN2 NeuronCores.

Sharding: batch*heads (32) split 4-heads-per-core across 8 cores; each core
computes full attention for its heads; the boolean mask is shared (broadcast
to every core). No cross-device communication.

Device algorithm (per head), computed in transposed "S^T" layout so the
softmax probabilities land with the contraction (k) dim on partitions and
feed the P@V matmul with no on-device transposes:

  S^T[k, q]  = K^T[d, k].T @ Q^T[d, q]          (PE, d=64 contraction,
                                                 consecutive k-blocks packed
                                                 in opposite PE row halves)
  P^T[k, q]  = exp(scale * S^T) * keepT[k, q]
  O^T_aug    = V_aug[k, d+1].T @ P^T[k, q]      (PE, accumulated over k in
                                                 PSUM; V_aug has a ones
                                                 column -> row d holds the
                                                 softmax denominators)

The exp+mask elementwise work is the bottleneck: 16.8M score elements per
core must leave PSUM through ACT (~1.43 ns/elem measured) or DVE
(~1.54 ns/elem), so it is split across three engines by a static per
(qh, k-block-pair) schedule:

  - "A" pairs: ACT exp LUT (exact), then a keepT fp16 tensor_tensor
    multiply on DVE (2x mode, ~0.89 ns/elem) or GPSIMD (~2.45 ns/elem).
  - "S2" pairs (Schraudolph, phase-averaged): one DVE scalar_tensor_tensor
    computes uint16(EA*s + ebmaskT) straight from PSUM.  For kept entries
    ebmaskT holds EB (so the uint16 bitcast as fp16 IS exp(scale*s)/2.34 up
    to a +-3% mantissa sawtooth); for masked entries it holds EB-30000,
    the result is negative, and the HW fp32->uint16 convert SATURATES TO
    ZERO -- masking is free.  xiB = xiA - 512 (4x-mode int add; saturates
    to 0 for masked; bitcast value = scaled exp/sqrt2 with the sawtooth
    phase shifted half a period); avg = yA + yB cancels the sawtooth's
    first harmonic (+-1.1% residual).  EB is chosen so the summed scale is
    exactly 1.0 relative to the A-path blocks (a per-block scale mismatch
    would NOT be absorbed by the softmax normalization).
  - "S1" pairs: single-sample version of the above -- one fused op total
    per e-half, +-3% sawtooth, so only ~4 pairs/core fit the error budget.

O^T_aug [65, 2048] fp32 is copied PSUM->SBUF (ACT) and DMA'd to HBM; the
final normalize (divide by denominator row) + un-transpose happens on host.

Softmax needs no running-max: scores*scale ~ N(0,1), |max| < ~7, exp() is
safely in range, and softmax is shift-invariant.
"""

import sys

for _p in ("/opt/trn_rl_repo", "/root/.axon_site/_ro/trn_rl_repo"):
    if _p not in sys.path:
        sys.path.append(_p)

import numpy as np
import ml_dtypes

B, H, S, D = 2, 16, 2048, 64
N_CORES = 8
HPC = (B * H) // N_CORES  # heads per core
P = 128
KB = S // P               # k blocks per head
QH = 2                    # q halves (elementwise tile free dim = S/QH)
QW = S // QH
SCALE = 1.0 / 8.0         # 1/sqrt(D)
LN2 = float(np.log(2.0))

QK_DTYPE = "float32r"     # "bfloat16" | "float32r" (QK matmul operand dtype)
DEFAULT_OPTS = ("fp16", "allact", "gps0", "nolag")

# Schraudolph fp16-bitcast exp constants.
EA16 = float(2.0**10 * SCALE / LN2)
# S1 (single-sample): value = bitcast_f16(u16(EA*s + EB_S1)) ~ exp(s/8),
# EB_S1 = 15*2^10 - 48 (sawtooth centering; exactly representable in fp16).
EB_S1 = 15312.0
# S2 "avtt" (two-sample average yA + bitcast(xiA-512) on DVE): the sum's
# scale is (1+1/sqrt2)*2^((EB_S2-15360)/1024); EB_S2 = 14520 makes it 1.0
# (+0.13%) so S2 blocks are consistent with the exact-exp A blocks in the
# same row (a per-block scale mismatch is NOT absorbed by softmax).
EB_S2 = 14520.0
# S2 default ("avpe"): both samples go to the PE directly -- yA against V,
# yB = bitcast(xiA-512) against sqrt2*V -- so the average costs no DVE op.
# 14281 centers the summed scale at 1.0 (numeric scan, +-0.80% residual).
EB_S2PE = 14281.0
EB_MASKED = -30000.0      # added to EB for masked entries -> u16 saturates to 0

# Static path schedule per (qh, kbp): "A" = ACT exact exp, "S2"/"S1" as
# above.  Shared by the device build and the host-side ebmask bake.
# S2/S1 pairs sit MID-instance: the PE queue is strictly in-order, so a
# tail S2 pair would head-of-line-block the next instance's QK matmuls on
# the DVE chain latency.  First/last pairs are always "A".
SCHED = {
    0: ("A", "A", "S2", "A", "A", "S2", "A", "A"),
    1: ("A", "S2", "A", "A", "S1", "A", "S2", "A"),
}
# keep/ebmask blocks are COMPACTED host-side: keepT[qh, 2*ai:2*ai+2] holds
# the k-block pair of the ai-th "A" pair of that qh (S1/S2 pairs don't read
# keep -- their masking is fused into the ebmask saturation); ebT[qh,
# 2*si:2*si+2] holds the si-th S1/S2 pair's blocks.
N_KEEP = 12
N_EB = 6
# PE software pipelining: the PE queue is strictly in-order, so PV matmuls
# are emitted PV_LAG k-block-pairs behind their QK matmuls -- a PV that
# waited inline on its pair's exp/mask chain would head-of-line-block all
# later QK work.
PV_LAG = 3

_CACHE = {}


def _parse_opts(opts):
    gps_pct = 25
    for o in opts:
        if o.startswith("gps"):
            gps_pct = int(o[3:])
    return gps_pct


def _sched(opts):
    """(schedule, n_keep): 'allact' = exact exp everywhere (ACT-bound but
    every engine then runs at its measured-model speed)."""
    if "allact" in opts:
        return {0: ("A",) * 8, 1: ("A",) * 8}, 16
    return SCHED, N_KEEP


def _build(repeats=1, qk_dtype=QK_DTYPE, opts=()):
    opts = frozenset(opts)
    gps_pct = _parse_opts(opts)
    avtt = "avtt" in opts
    sched, n_keep = _sched(opts)
    pv_lag = 0 if "nolag" in opts else PV_LAG
    import concourse.mybir as mybir
    import concourse.tile as tile
    from concourse import bacc

    dt = mybir.dt
    qk_dt = getattr(dt, qk_dtype)
    half_dt = dt.float16 if "fp16" in opts else dt.bfloat16
    nc = bacc.Bacc(
        "TRN2", target_bir_lowering=False, debug=False, num_devices=N_CORES
    )

    g3 = "g3" in opts
    qT = nc.dram_tensor("qT", [HPC, D, S], qk_dt, kind="ExternalInput").ap()
    kT = nc.dram_tensor("kT", [HPC, D, S], qk_dt, kind="ExternalInput").ap()
    v = nc.dram_tensor("v", [HPC, S, D], half_dt, kind="ExternalInput").ap()
    vB = nc.dram_tensor("vB", [HPC, S, D], half_dt, kind="ExternalInput").ap()
    keep_shape = [QH, 32, P, 512] if g3 else [QH, n_keep, P, QW]
    keepT = nc.dram_tensor(
        "keepT", keep_shape, half_dt, kind="ExternalInput"
    ).ap()
    ebT = nc.dram_tensor(
        "ebT", [QH, N_EB, P, QW], half_dt, kind="ExternalInput"
    ).ap()
    out = nc.dram_tensor(
        "out", [HPC, D + 1, S], dt.float32, kind="ExternalOutput"
    ).ap()

    Exp = mybir.ActivationFunctionType.Exp
    mult = mybir.AluOpType.mult
    add = mybir.AluOpType.add

    with tile.TileContext(nc) as tc:
        with (
            tc.tile_pool(name="keep_pool", bufs=1) as keep_pool,
            tc.tile_pool(name="qk_pool", bufs=2) as qk_pool,
            tc.tile_pool(name="v_pool", bufs=2) as v_pool,
            tc.tile_pool(name="p_pool", bufs=3) as p_pool,
            tc.tile_pool(name="pm_pool", bufs=PV_LAG + 2) as pm_pool,
            tc.tile_pool(name="x_pool", bufs=4) as x_pool,
            tc.tile_pool(name="ob_pool", bufs=1) as ob_pool,
            tc.tile_pool(name="s_psum", bufs=(2 if g3 else 3),
                         space="PSUM") as s_psum,
            tc.tile_pool(name="o_psum", bufs=1, space="PSUM") as o_psum,
        ):
            # keep: only the A-pair blocks (kb < 2*N_KEEP/2 per qh);
            # [P, QH, N_KEEP, QW] so each pair-slice is contiguous (keeps
            # the DVE mask tensor_tensor in 2x packed mode).  In g3 mode the
            # layout is chunk-indexed [P, QH, 32, 512] (QK emission order).
            if g3:
                keep_sb = keep_pool.tile([P, QH, 32, 512], half_dt)
            else:
                keep_sb = keep_pool.tile([P, QH, n_keep, QW], half_dt)
            for qh in range(QH):
                nc.sync.dma_start(
                    out=keep_sb[:, qh, :, :],
                    in_=keepT[qh].rearrange("kb p qw -> p kb qw"),
                )
            # ebmask: per-(S-pair) EB constant minus 30000*mask, compacted.
            eb_sb = keep_pool.tile([P, QH, N_EB, QW], half_dt)
            for qh in range(QH):
                nc.sync.dma_start(
                    out=eb_sb[:, qh, :, :],
                    in_=ebT[qh].rearrange("kb p qw -> p kb qw"),
                )

            gps_acc = [0]

            def mask_engine():
                gps_acc[0] += gps_pct
                if gps_acc[0] >= 100:
                    gps_acc[0] -= 100
                    return nc.gpsimd
                return nc.vector

            def body(rep):
                for h in range(HPC):
                    qTr = qk_pool.tile([P, S], qk_dt, tag="qTr", name=f"qTr_{h}")
                    kTr = qk_pool.tile([P, S], qk_dt, tag="kTr", name=f"kTr_{h}")
                    for half in (0, 1):
                        nc.sync.dma_start(
                            out=qTr[half * 64:(half + 1) * 64, :], in_=qT[h]
                        )
                        nc.sync.dma_start(
                            out=kTr[half * 64:(half + 1) * 64, :], in_=kT[h]
                        )

                    v_sb = v_pool.tile(
                        [P, KB, D + 1], half_dt, tag="v", name=f"v_{h}"
                    )
                    v_re = v[h].rearrange("(kb p) d -> p kb d", p=P)
                    nc.sync.dma_start(out=v_sb[:, :, 0:D], in_=v_re)
                    nc.gpsimd.memset(v_sb[:, :, D:D + 1], 1.0)
                    if not avtt and any("S2" in sched[_q] for _q in sched):
                        # sqrt2*V for the second Schraudolph sample; its
                        # ones column is sqrt2 so the denominators get the
                        # same equal-weight average as the value rows.
                        vB_sb = v_pool.tile(
                            [P, KB, D + 1], half_dt, tag="v2", name=f"v2_{h}"
                        )
                        vB_re = vB[h].rearrange("(kb p) d -> p kb d", p=P)
                        nc.sync.dma_start(out=vB_sb[:, :, 0:D], in_=vB_re)
                        nc.gpsimd.memset(
                            vB_sb[:, :, D:D + 1], float(np.sqrt(2.0))
                        )

                    if g3:
                        # grouped-exp path: 32 [128,512] score chunks per
                        # (h, qh), exp'd in groups of 3 (FD=1536) to
                        # amortize the ~600ns per-ACT-op overhead.
                        for qh in range(QH):
                            ot = o_psum.tile(
                                [D + 1, QW], dt.float32, tag="ot",
                                name=f"ot_{h}_{qh}",
                            )
                            chunk_src = {}
                            pv_kb = 0
                            for g in range((32 + 2) // 3):
                                c0, c1 = 3 * g, min(3 * g + 3, 32)
                                n = c1 - c0
                                s3 = s_psum.tile(
                                    [P, 3, 512], dt.float32, tag="s3",
                                    name=f"s3_{h}_{qh}_{g}",
                                )
                                for c in range(c0, c1):
                                    kbp, e, qc = c // 4, (c % 4) // 2, c % 2
                                    kb = 2 * kbp + e
                                    half = 64 * e
                                    q0 = qh * QW + qc * 512
                                    nc.tensor.matmul(
                                        s3[:, c - c0, :],
                                        lhsT=kTr[half:half + 64,
                                                 kb * P:(kb + 1) * P],
                                        rhs=qTr[half:half + 64, q0:q0 + 512],
                                        start=True,
                                        stop=True,
                                    )
                                p3 = p_pool.tile(
                                    [P, 3, 512], half_dt, tag="p3",
                                    name=f"p3_{h}_{qh}_{g}",
                                )
                                nc.scalar.activation(
                                    p3[:, 0:n, :], s3[:, 0:n, :], Exp,
                                    scale=SCALE,
                                )
                                pm3 = pm_pool.tile(
                                    [P, 3, 512], half_dt, tag="pm3",
                                    name=f"pm3_{h}_{qh}_{g}",
                                )
                                mask_engine().tensor_tensor(
                                    pm3[:, 0:n, :],
                                    p3[:, 0:n, :],
                                    keep_sb[:, qh, c0:c1, :],
                                    mult,
                                )
                                for c in range(c0, c1):
                                    chunk_src[c] = (pm3, c - c0)
                                while pv_kb < KB:
                                    kbp, e = pv_kb // 2, pv_kb % 2
                                    cq1 = 4 * kbp + 2 * e + 1
                                    if cq1 >= c1:
                                        break
                                    for qc in (0, 1):
                                        t, sl = chunk_src[cq1 - 1 + qc]
                                        nc.tensor.matmul(
                                            ot[:, qc * 512:(qc + 1) * 512],
                                            lhsT=v_sb[:, pv_kb, :],
                                            rhs=t[:, sl, :],
                                            start=(pv_kb == 0),
                                            stop=(pv_kb == KB - 1),
                                        )
                                    pv_kb += 1
                            ob_sb = ob_pool.tile(
                                [D + 1, QW], dt.float32, tag="ob",
                                name=f"ob_{h}_{qh}",
                            )
                            if "ob_dve" in opts:
                                nc.vector.tensor_copy(ob_sb[:, :], ot[:, :])
                            else:
                                nc.scalar.copy(ob_sb[:, :], ot[:, :])
                            nc.sync.dma_start(
                                out=out[h][:, qh * QW:(qh + 1) * QW],
                                in_=ob_sb[:, :],
                            )
                        continue

                    for qh in range(QH):
                        a_idx = 0
                        s_idx = 0
                        pv_q = []
                        ot = o_psum.tile(
                            [D + 1, QW], dt.float32, tag="ot", name=f"ot_{h}_{qh}"
                        )

                        def emit_pv(kbp, streams):
                            for w_sb, src in streams:
                                for e in (0, 1):
                                    kb = 2 * kbp + e
                                    for qc in range(QW // 512):
                                        nc.tensor.matmul(
                                            ot[:, qc * 512:(qc + 1) * 512],
                                            lhsT=w_sb[:, kb, :],
                                            rhs=src[:, e, qc * 512:(qc + 1) * 512],
                                            start=(kb == 0),
                                            stop=(kb == KB - 1),
                                        )

                        for kbp in range(KB // 2):
                            path = sched[qh][kbp]
                            if path == "A":
                                p2 = p_pool.tile(
                                    [P, 2, QW], half_dt, tag="p",
                                    name=f"p_{h}_{qh}_{kbp}",
                                )
                            else:
                                xiA = x_pool.tile(
                                    [P, 2, QW], dt.uint16, tag="xi",
                                    name=f"xi_{h}_{qh}_{kbp}",
                                )
                            for e in (0, 1):
                                kb = 2 * kbp + e
                                half = 64 * e
                                s_ps = s_psum.tile(
                                    [P, QW], dt.float32, tag="s",
                                    name=f"s_{h}_{qh}_{kb}",
                                )
                                for qc in range(QW // 512):
                                    q0 = qh * QW + qc * 512
                                    nc.tensor.matmul(
                                        s_ps[:, qc * 512:(qc + 1) * 512],
                                        lhsT=kTr[half:half + 64, kb * P:(kb + 1) * P],
                                        rhs=qTr[half:half + 64, q0:q0 + 512],
                                        start=True,
                                        stop=True,
                                    )
                                if path == "A":
                                    nc.scalar.activation(
                                        p2[:, e, :], s_ps[:, :], Exp, scale=SCALE
                                    )
                                else:
                                    # fused exp-approx + mask: masked entries
                                    # go negative and saturate to u16 zero.
                                    nc.vector.scalar_tensor_tensor(
                                        xiA[:, e, :], s_ps[:, :], EA16,
                                        eb_sb[:, qh, 2 * s_idx + e, :],
                                        mult, add,
                                    )
                            streams = None
                            if path == "A":
                                pm2 = pm_pool.tile(
                                    [P, 2, QW], half_dt, tag="pm",
                                    name=f"pm_{h}_{qh}_{kbp}",
                                )
                                mask_engine().tensor_tensor(
                                    pm2[:, :, :],
                                    p2[:, :, :],
                                    keep_sb[:, qh, 2 * a_idx:2 * a_idx + 2, :],
                                    mult,
                                )
                                a_idx += 1
                                streams = [(v_sb, pm2)]
                            elif path == "S2":
                                xiB = x_pool.tile(
                                    [P, 2, QW], dt.uint16, tag="xj",
                                    name=f"xj_{h}_{qh}_{kbp}",
                                )
                                # -512: half-period sawtooth phase shift and
                                # a 1/sqrt2 value scale; masked entries
                                # (xiA=0) re-saturate to 0.
                                nc.vector.tensor_scalar_add(
                                    xiB[:, :, :], xiA[:, :, :], -512
                                )
                                if avtt:
                                    av2 = p_pool.tile(
                                        [P, 2, QW], half_dt, tag="av",
                                        name=f"av_{h}_{qh}_{kbp}",
                                    )
                                    nc.vector.tensor_tensor(
                                        av2[:, :, :],
                                        xiA.bitcast(half_dt)[:, :, :],
                                        xiB.bitcast(half_dt)[:, :, :],
                                        add,
                                    )
                                    streams = [(v_sb, av2)]
                                else:
                                    # the average happens inside the PE
                                    # accumulation: yA against V, yB
                                    # against sqrt2*V (S2 pairs are never
                                    # at kb 0 or KB-1, so start/stop flags
                                    # stay on the A-path matmuls).
                                    streams = [
                                        (v_sb, xiA.bitcast(half_dt)),
                                        (vB_sb, xiB.bitcast(half_dt)),
                                    ]
                                s_idx += 1
                            else:  # S1
                                streams = [(v_sb, xiA.bitcast(half_dt))]
                                s_idx += 1
                            pv_q.append((kbp, streams))
                            if len(pv_q) > pv_lag:
                                emit_pv(*pv_q.pop(0))
                        for item in pv_q:
                            emit_pv(*item)

                        ob_sb = ob_pool.tile(
                            [D + 1, QW], dt.float32, tag="ob", name=f"ob_{h}_{qh}"
                        )
                        if "ob_dve" in opts:
                            nc.vector.tensor_copy(ob_sb[:, :], ot[:, :])
                        else:
                            nc.scalar.copy(ob_sb[:, :], ot[:, :])
                        nc.sync.dma_start(
                            out=out[h][:, qh * QW:(qh + 1) * QW], in_=ob_sb[:, :]
                        )

            if repeats == 1:
                body(0)
            else:
                with tc.For_i(
                    0, repeats, 1,
                    hint_engines=(
                        mybir.EngineType.PE,
                        mybir.EngineType.DVE,
                        mybir.EngineType.Activation,
                        mybir.EngineType.Pool,
                    ),
                ):
                    body(0)

    nc.compile()
    return nc


def get_nc(repeats=1, qk_dtype=QK_DTYPE, opts=()):
    key = ("nc", repeats, qk_dtype, frozenset(opts))
    if key not in _CACHE:
        _CACHE[key] = _build(repeats, qk_dtype, opts)
    return _CACHE[key]


def prep_in_maps(q, k, v, mask, qk_dtype=QK_DTYPE, half="float16",
                 opts=DEFAULT_OPTS):
    avtt = "avtt" in opts
    sched, n_keep = _sched(opts)
    bf16 = np.float16 if half == "float16" else ml_dtypes.bfloat16
    qk_np = np.float32 if qk_dtype == "float32r" else bf16
    q = np.asarray(q, dtype=np.float32).reshape(B * H, S, D)
    k = np.asarray(k, dtype=np.float32).reshape(B * H, S, D)
    vv = np.asarray(v, dtype=np.float32).reshape(B * H, S, D)
    mask = np.asarray(mask).reshape(S, S)
    keepT = np.ascontiguousarray((1 - mask).T.astype(np.float32))  # [k, q]
    k4 = keepT.reshape(KB, P, QH, QW).transpose(2, 0, 1, 3)  # [QH, KB, P, QW]
    if "g3" in opts:
        # chunk-indexed keep layout matching the g3 QK emission order.
        keep4 = np.zeros((QH, 32, P, 512), dtype=np.float32)
        for qh in range(QH):
            for c in range(32):
                kbp, e, qc = c // 4, (c % 4) // 2, c % 2
                kb = 2 * kbp + e
                keep4[qh, c] = k4[qh, kb, :, qc * 512:(qc + 1) * 512]
        keep4 = np.ascontiguousarray(keep4).astype(bf16)
        eb4 = np.zeros((QH, N_EB, P, QW), dtype=bf16)
        in_maps = []
        for c in range(N_CORES):
            sl = slice(c * HPC, (c + 1) * HPC)
            in_maps.append({
                "qT": np.ascontiguousarray(
                    q[sl].transpose(0, 2, 1)).astype(qk_np),
                "kT": np.ascontiguousarray(
                    k[sl].transpose(0, 2, 1)).astype(qk_np),
                "v": vv[sl].astype(bf16),
                "vB": (vv[sl] * np.sqrt(2.0)).astype(bf16),
                "keepT": keep4,
                "ebT": eb4,
            })
        return in_maps
    # compacted keep (A pairs) and ebmask (S1/S2 pairs), in pair order.
    keep4 = np.zeros((QH, n_keep, P, QW), dtype=np.float32)
    eb4 = np.zeros((QH, N_EB, P, QW), dtype=np.float32)
    eb_s2 = EB_S2 if avtt else EB_S2PE
    for qh in range(QH):
        ai = 0
        si = 0
        for kbp in range(KB // 2):
            pth = sched[qh][kbp]
            blk = k4[qh, 2 * kbp:2 * kbp + 2]
            if pth == "A":
                keep4[qh, 2 * ai:2 * ai + 2] = blk
                ai += 1
            else:
                ebv = EB_S1 if pth == "S1" else eb_s2
                eb4[qh, 2 * si:2 * si + 2] = ebv + EB_MASKED * (1.0 - blk)
                si += 1
    keep4 = np.ascontiguousarray(keep4).astype(bf16)
    eb4 = np.ascontiguousarray(eb4).astype(bf16)
    in_maps = []
    for c in range(N_CORES):
        sl = slice(c * HPC, (c + 1) * HPC)
        in_maps.append({
            "qT": np.ascontiguousarray(q[sl].transpose(0, 2, 1)).astype(qk_np),
            "kT": np.ascontiguousarray(k[sl].transpose(0, 2, 1)).astype(qk_np),
            "v": vv[sl].astype(bf16),
            "vB": (vv[sl] * np.sqrt(2.0)).astype(bf16),
            "keepT": keep4,
            "ebT": eb4,
        })
    return in_maps


def finish_output(core_results):
    """core_results: list of [HPC, D+1, S] fp32 arrays -> [B, H, S, D] fp32."""
    outs = []
    for r in core_results:
        r = np.asarray(r, dtype=np.float32)
        o = (r[:, :D, :] / r[:, D:D + 1, :]).transpose(0, 2, 1)
        outs.append(o)
    return np.concatenate(outs, axis=0).reshape(B, H, S, D).astype(np.float32)


def kernel(q, k, v, mask):
    from concourse import bass_utils

    nc = get_nc(1, opts=DEFAULT_OPTS)
    in_maps = prep_in_maps(q, k, v, mask, opts=DEFAULT_OPTS)
    bkr = bass_utils.run_bass_kernel_spmd(nc, in_maps, list(range(N_CORES)))
    return finish_output([bkr.results[c]["out"] for c in range(N_CORES)])


# revision 48
# speedup vs baseline: 1.0942x; 1.0942x over previous
"""Masked multi-head attention (B=2, H=16, S=2048, D=64) on 8 TRN2 NeuronCores.

Sharding: batch*heads (32) split 4-heads-per-core across 8 cores; each core
computes full attention for its heads; the boolean mask is shared (broadcast
to every core). No cross-device communication.

Device algorithm (per head), computed in transposed "S^T" layout so the
softmax probabilities land with the contraction (k) dim on partitions and
feed the P@V matmul with no on-device transposes:

  S^T[k, q]  = K^T[d, k].T @ Q^T[d, q]          (PE, d=64 contraction,
                                                 consecutive k-blocks packed
                                                 in opposite PE row halves)
  P^T[k, q]  = exp(scale * S^T) * keepT[k, q]
  O^T_aug    = V_aug[k, d+1].T @ P^T[k, q]      (PE, accumulated over k in
                                                 PSUM; V_aug has a ones
                                                 column -> row d holds the
                                                 softmax denominators)

The exp+mask elementwise work is the bottleneck: 16.8M score elements per
core must leave PSUM through ACT (~1.43 ns/elem measured, flat rate) or
DVE.  The DEFAULT config ("allact") runs every block through the ACT exp:
hardware A/B runs showed that any DVE-offload of the PSUM extraction slows
the kernel down (ACT and DVE PSUM reads appear to serialize against each
other), so the exact-exp ACT path everywhere is both the fastest measured
configuration AND the most accurate (rel err 5e-4).  The alternative
paths below remain selectable via opts:

  - "A" pairs: ACT exp LUT (exact), then a keepT fp16 tensor_tensor
    multiply on DVE (2x mode, ~0.89 ns/elem) or GPSIMD (~2.45 ns/elem).
  - "S2" pairs (Schraudolph, phase-averaged): one DVE scalar_tensor_tensor
    computes uint16(EA*s + ebmaskT) straight from PSUM.  For kept entries
    ebmaskT holds EB (so the uint16 bitcast as fp16 IS exp(scale*s)/2.34 up
    to a +-3% mantissa sawtooth); for masked entries it holds EB-30000,
    the result is negative, and the HW fp32->uint16 convert SATURATES TO
    ZERO -- masking is free.  xiB = xiA - 512 (4x-mode int add; saturates
    to 0 for masked; bitcast value = scaled exp/sqrt2 with the sawtooth
    phase shifted half a period); avg = yA + yB cancels the sawtooth's
    first harmonic (+-1.1% residual).  EB is chosen so the summed scale is
    exactly 1.0 relative to the A-path blocks (a per-block scale mismatch
    would NOT be absorbed by the softmax normalization).
  - "S1" pairs: single-sample version of the above -- one fused op total
    per e-half, +-3% sawtooth, so only ~4 pairs/core fit the error budget.

O^T_aug [65, 2048] fp32 is copied PSUM->SBUF (ACT) and DMA'd to HBM; the
final normalize (divide by denominator row) + un-transpose happens on host.

Softmax needs no running-max: scores*scale ~ N(0,1), |max| < ~7, exp() is
safely in range, and softmax is shift-invariant.
"""

import sys

for _p in ("/opt/trn_rl_repo", "/root/.axon_site/_ro/trn_rl_repo"):
    if _p not in sys.path:
        sys.path.append(_p)

import numpy as np
import ml_dtypes

B, H, S, D = 2, 16, 2048, 64
N_CORES = 8
HPC = (B * H) // N_CORES  # heads per core
P = 128
KB = S // P               # k blocks per head
QH = 2                    # q halves (elementwise tile free dim = S/QH)
QW = S // QH
SCALE = 1.0 / 8.0         # 1/sqrt(D)
LN2 = float(np.log(2.0))

QK_DTYPE = "float32r"     # "bfloat16" | "float32r" (QK matmul operand dtype)
DEFAULT_OPTS = ("fp16", "allact", "gps0", "nolag")

# Schraudolph fp16-bitcast exp constants.
EA16 = float(2.0**10 * SCALE / LN2)
# S1 (single-sample): value = bitcast_f16(u16(EA*s + EB_S1)) ~ exp(s/8),
# EB_S1 = 15*2^10 - 48 (sawtooth centering; exactly representable in fp16).
EB_S1 = 15312.0
# S2 "avtt" (two-sample average yA + bitcast(xiA-512) on DVE): the sum's
# scale is (1+1/sqrt2)*2^((EB_S2-15360)/1024); EB_S2 = 14520 makes it 1.0
# (+0.13%) so S2 blocks are consistent with the exact-exp A blocks in the
# same row (a per-block scale mismatch is NOT absorbed by softmax).
EB_S2 = 14520.0
# S2 default ("avpe"): both samples go to the PE directly -- yA against V,
# yB = bitcast(xiA-512) against sqrt2*V -- so the average costs no DVE op.
# 14281 centers the summed scale at 1.0 (numeric scan, +-0.80% residual).
EB_S2PE = 14281.0
EB_MASKED = -30000.0      # added to EB for masked entries -> u16 saturates to 0

# Static path schedule per (qh, kbp): "A" = ACT exact exp, "S2"/"S1" as
# above.  Shared by the device build and the host-side ebmask bake.
# S2/S1 pairs sit MID-instance: the PE queue is strictly in-order, so a
# tail S2 pair would head-of-line-block the next instance's QK matmuls on
# the DVE chain latency.  First/last pairs are always "A".
SCHED = {
    0: ("A", "A", "S2", "A", "A", "S2", "A", "A"),
    1: ("A", "S2", "A", "A", "S1", "A", "S2", "A"),
}
# keep/ebmask blocks are COMPACTED host-side: keepT[qh, 2*ai:2*ai+2] holds
# the k-block pair of the ai-th "A" pair of that qh (S1/S2 pairs don't read
# keep -- their masking is fused into the ebmask saturation); ebT[qh,
# 2*si:2*si+2] holds the si-th S1/S2 pair's blocks.
N_KEEP = 12
N_EB = 6
# PE software pipelining: the PE queue is strictly in-order, so PV matmuls
# are emitted PV_LAG k-block-pairs behind their QK matmuls -- a PV that
# waited inline on its pair's exp/mask chain would head-of-line-block all
# later QK work.
PV_LAG = 3

_CACHE = {}


def _parse_opts(opts):
    gps_pct = 25
    for o in opts:
        if o.startswith("gps"):
            gps_pct = int(o[3:])
    return gps_pct


def _sched(opts):
    """(schedule, n_keep): 'allact' = exact exp everywhere (ACT-bound but
    every engine then runs at its measured-model speed)."""
    if "allact" in opts:
        return {0: ("A",) * 8, 1: ("A",) * 8}, 16
    return SCHED, N_KEEP


def _build(repeats=1, qk_dtype=QK_DTYPE, opts=()):
    opts = frozenset(opts)
    gps_pct = _parse_opts(opts)
    avtt = "avtt" in opts
    sched, n_keep = _sched(opts)
    pv_lag = 0 if "nolag" in opts else PV_LAG
    import concourse.mybir as mybir
    import concourse.tile as tile
    from concourse import bacc

    dt = mybir.dt
    qk_dt = getattr(dt, qk_dtype)
    half_dt = dt.float16 if "fp16" in opts else dt.bfloat16
    nc = bacc.Bacc(
        "TRN2", target_bir_lowering=False, debug=False, num_devices=N_CORES
    )

    g3 = "g3" in opts
    qT = nc.dram_tensor("qT", [HPC, D, S], qk_dt, kind="ExternalInput").ap()
    kT = nc.dram_tensor("kT", [HPC, D, S], qk_dt, kind="ExternalInput").ap()
    v = nc.dram_tensor("v", [HPC, S, D], half_dt, kind="ExternalInput").ap()
    vB = nc.dram_tensor("vB", [HPC, S, D], half_dt, kind="ExternalInput").ap()
    keep_shape = [QH, 32, P, 512] if g3 else [QH, n_keep, P, QW]
    keepT = nc.dram_tensor(
        "keepT", keep_shape, half_dt, kind="ExternalInput"
    ).ap()
    ebT = nc.dram_tensor(
        "ebT", [QH, N_EB, P, QW], half_dt, kind="ExternalInput"
    ).ap()
    out = nc.dram_tensor(
        "out", [HPC, D + 1, S], dt.float32, kind="ExternalOutput"
    ).ap()

    Exp = mybir.ActivationFunctionType.Exp
    mult = mybir.AluOpType.mult
    add = mybir.AluOpType.add

    with tile.TileContext(nc) as tc:
        with (
            tc.tile_pool(name="keep_pool", bufs=1) as keep_pool,
            tc.tile_pool(name="qk_pool", bufs=2) as qk_pool,
            tc.tile_pool(name="v_pool", bufs=2) as v_pool,
            tc.tile_pool(name="p_pool", bufs=3) as p_pool,
            tc.tile_pool(name="pm_pool", bufs=PV_LAG + 2) as pm_pool,
            tc.tile_pool(name="x_pool", bufs=4) as x_pool,
            tc.tile_pool(name="ob_pool", bufs=1) as ob_pool,
            tc.tile_pool(name="s_psum", bufs=(2 if g3 else 3),
                         space="PSUM") as s_psum,
            tc.tile_pool(name="o_psum", bufs=1, space="PSUM") as o_psum,
        ):
            # keep: only the A-pair blocks (kb < 2*N_KEEP/2 per qh);
            # [P, QH, N_KEEP, QW] so each pair-slice is contiguous (keeps
            # the DVE mask tensor_tensor in 2x packed mode).  In g3 mode the
            # layout is chunk-indexed [P, QH, 32, 512] (QK emission order).
            if g3:
                keep_sb = keep_pool.tile([P, QH, 32, 512], half_dt)
            else:
                keep_sb = keep_pool.tile([P, QH, n_keep, QW], half_dt)
            for qh in range(QH):
                nc.sync.dma_start(
                    out=keep_sb[:, qh, :, :],
                    in_=keepT[qh].rearrange("kb p qw -> p kb qw"),
                )
            # ebmask: per-(S-pair) EB constant minus 30000*mask, compacted.
            eb_sb = keep_pool.tile([P, QH, N_EB, QW], half_dt)
            for qh in range(QH):
                nc.sync.dma_start(
                    out=eb_sb[:, qh, :, :],
                    in_=ebT[qh].rearrange("kb p qw -> p kb qw"),
                )

            gps_acc = [0]

            def mask_engine():
                gps_acc[0] += gps_pct
                if gps_acc[0] >= 100:
                    gps_acc[0] -= 100
                    return nc.gpsimd
                return nc.vector

            def body(rep):
                for h in range(HPC):
                    qTr = qk_pool.tile([P, S], qk_dt, tag="qTr", name=f"qTr_{h}")
                    kTr = qk_pool.tile([P, S], qk_dt, tag="kTr", name=f"kTr_{h}")
                    for half in (0, 1):
                        nc.sync.dma_start(
                            out=qTr[half * 64:(half + 1) * 64, :], in_=qT[h]
                        )
                        nc.sync.dma_start(
                            out=kTr[half * 64:(half + 1) * 64, :], in_=kT[h]
                        )

                    v_sb = v_pool.tile(
                        [P, KB, D + 1], half_dt, tag="v", name=f"v_{h}"
                    )
                    v_re = v[h].rearrange("(kb p) d -> p kb d", p=P)
                    nc.sync.dma_start(out=v_sb[:, :, 0:D], in_=v_re)
                    nc.gpsimd.memset(v_sb[:, :, D:D + 1], 1.0)
                    if not avtt and any("S2" in sched[_q] for _q in sched):
                        # sqrt2*V for the second Schraudolph sample; its
                        # ones column is sqrt2 so the denominators get the
                        # same equal-weight average as the value rows.
                        vB_sb = v_pool.tile(
                            [P, KB, D + 1], half_dt, tag="v2", name=f"v2_{h}"
                        )
                        vB_re = vB[h].rearrange("(kb p) d -> p kb d", p=P)
                        nc.sync.dma_start(out=vB_sb[:, :, 0:D], in_=vB_re)
                        nc.gpsimd.memset(
                            vB_sb[:, :, D:D + 1], float(np.sqrt(2.0))
                        )

                    if g3:
                        # grouped-exp path: 32 [128,512] score chunks per
                        # (h, qh), exp'd in groups of 3 (FD=1536) to
                        # amortize the ~600ns per-ACT-op overhead.
                        for qh in range(QH):
                            ot = o_psum.tile(
                                [D + 1, QW], dt.float32, tag="ot",
                                name=f"ot_{h}_{qh}",
                            )
                            chunk_src = {}
                            pv_kb = 0
                            for g in range((32 + 2) // 3):
                                c0, c1 = 3 * g, min(3 * g + 3, 32)
                                n = c1 - c0
                                s3 = s_psum.tile(
                                    [P, 3, 512], dt.float32, tag="s3",
                                    name=f"s3_{h}_{qh}_{g}",
                                )
                                for c in range(c0, c1):
                                    kbp, e, qc = c // 4, (c % 4) // 2, c % 2
                                    kb = 2 * kbp + e
                                    half = 64 * e
                                    q0 = qh * QW + qc * 512
                                    nc.tensor.matmul(
                                        s3[:, c - c0, :],
                                        lhsT=kTr[half:half + 64,
                                                 kb * P:(kb + 1) * P],
                                        rhs=qTr[half:half + 64, q0:q0 + 512],
                                        start=True,
                                        stop=True,
                                    )
                                p3 = p_pool.tile(
                                    [P, 3, 512], half_dt, tag="p3",
                                    name=f"p3_{h}_{qh}_{g}",
                                )
                                nc.scalar.activation(
                                    p3[:, 0:n, :], s3[:, 0:n, :], Exp,
                                    scale=SCALE,
                                )
                                pm3 = pm_pool.tile(
                                    [P, 3, 512], half_dt, tag="pm3",
                                    name=f"pm3_{h}_{qh}_{g}",
                                )
                                mask_engine().tensor_tensor(
                                    pm3[:, 0:n, :],
                                    p3[:, 0:n, :],
                                    keep_sb[:, qh, c0:c1, :],
                                    mult,
                                )
                                for c in range(c0, c1):
                                    chunk_src[c] = (pm3, c - c0)
                                while pv_kb < KB:
                                    kbp, e = pv_kb // 2, pv_kb % 2
                                    cq1 = 4 * kbp + 2 * e + 1
                                    if cq1 >= c1:
                                        break
                                    for qc in (0, 1):
                                        t, sl = chunk_src[cq1 - 1 + qc]
                                        nc.tensor.matmul(
                                            ot[:, qc * 512:(qc + 1) * 512],
                                            lhsT=v_sb[:, pv_kb, :],
                                            rhs=t[:, sl, :],
                                            start=(pv_kb == 0),
                                            stop=(pv_kb == KB - 1),
                                        )
                                    pv_kb += 1
                            ob_sb = ob_pool.tile(
                                [D + 1, QW], dt.float32, tag="ob",
                                name=f"ob_{h}_{qh}",
                            )
                            if "ob_dve" in opts:
                                nc.vector.tensor_copy(ob_sb[:, :], ot[:, :])
                            else:
                                nc.scalar.copy(ob_sb[:, :], ot[:, :])
                            nc.sync.dma_start(
                                out=out[h][:, qh * QW:(qh + 1) * QW],
                                in_=ob_sb[:, :],
                            )
                        continue

                    for qh in range(QH):
                        a_idx = 0
                        s_idx = 0
                        pv_q = []
                        ot = o_psum.tile(
                            [D + 1, QW], dt.float32, tag="ot", name=f"ot_{h}_{qh}"
                        )

                        def emit_pv(kbp, streams):
                            for w_sb, src in streams:
                                for e in (0, 1):
                                    kb = 2 * kbp + e
                                    for qc in range(QW // 512):
                                        nc.tensor.matmul(
                                            ot[:, qc * 512:(qc + 1) * 512],
                                            lhsT=w_sb[:, kb, :],
                                            rhs=src[:, e, qc * 512:(qc + 1) * 512],
                                            start=(kb == 0),
                                            stop=(kb == KB - 1),
                                        )

                        for kbp in range(KB // 2):
                            path = sched[qh][kbp]
                            if path == "A":
                                p2 = p_pool.tile(
                                    [P, 2, QW], half_dt, tag="p",
                                    name=f"p_{h}_{qh}_{kbp}",
                                )
                            else:
                                xiA = x_pool.tile(
                                    [P, 2, QW], dt.uint16, tag="xi",
                                    name=f"xi_{h}_{qh}_{kbp}",
                                )
                            for e in (0, 1):
                                kb = 2 * kbp + e
                                half = 64 * e
                                s_ps = s_psum.tile(
                                    [P, QW], dt.float32, tag="s",
                                    name=f"s_{h}_{qh}_{kb}",
                                )
                                for qc in range(QW // 512):
                                    q0 = qh * QW + qc * 512
                                    nc.tensor.matmul(
                                        s_ps[:, qc * 512:(qc + 1) * 512],
                                        lhsT=kTr[half:half + 64, kb * P:(kb + 1) * P],
                                        rhs=qTr[half:half + 64, q0:q0 + 512],
                                        start=True,
                                        stop=True,
                                    )
                                if path == "A":
                                    nc.scalar.activation(
                                        p2[:, e, :], s_ps[:, :], Exp, scale=SCALE
                                    )
                                else:
                                    # fused exp-approx + mask: masked entries
                                    # go negative and saturate to u16 zero.
                                    nc.vector.scalar_tensor_tensor(
                                        xiA[:, e, :], s_ps[:, :], EA16,
                                        eb_sb[:, qh, 2 * s_idx + e, :],
                                        mult, add,
                                    )
                            streams = None
                            if path == "A":
                                pm2 = pm_pool.tile(
                                    [P, 2, QW], half_dt, tag="pm",
                                    name=f"pm_{h}_{qh}_{kbp}",
                                )
                                mask_engine().tensor_tensor(
                                    pm2[:, :, :],
                                    p2[:, :, :],
                                    keep_sb[:, qh, 2 * a_idx:2 * a_idx + 2, :],
                                    mult,
                                )
                                a_idx += 1
                                streams = [(v_sb, pm2)]
                            elif path == "S2":
                                xiB = x_pool.tile(
                                    [P, 2, QW], dt.uint16, tag="xj",
                                    name=f"xj_{h}_{qh}_{kbp}",
                                )
                                # -512: half-period sawtooth phase shift and
                                # a 1/sqrt2 value scale; masked entries
                                # (xiA=0) re-saturate to 0.
                                nc.vector.tensor_scalar_add(
                                    xiB[:, :, :], xiA[:, :, :], -512
                                )
                                if avtt:
                                    av2 = p_pool.tile(
                                        [P, 2, QW], half_dt, tag="av",
                                        name=f"av_{h}_{qh}_{kbp}",
                                    )
                                    nc.vector.tensor_tensor(
                                        av2[:, :, :],
                                        xiA.bitcast(half_dt)[:, :, :],
                                        xiB.bitcast(half_dt)[:, :, :],
                                        add,
                                    )
                                    streams = [(v_sb, av2)]
                                else:
                                    # the average happens inside the PE
                                    # accumulation: yA against V, yB
                                    # against sqrt2*V (S2 pairs are never
                                    # at kb 0 or KB-1, so start/stop flags
                                    # stay on the A-path matmuls).
                                    streams = [
                                        (v_sb, xiA.bitcast(half_dt)),
                                        (vB_sb, xiB.bitcast(half_dt)),
                                    ]
                                s_idx += 1
                            else:  # S1
                                streams = [(v_sb, xiA.bitcast(half_dt))]
                                s_idx += 1
                            pv_q.append((kbp, streams))
                            if len(pv_q) > pv_lag:
                                emit_pv(*pv_q.pop(0))
                        for item in pv_q:
                            emit_pv(*item)

                        ob_sb = ob_pool.tile(
                            [D + 1, QW], dt.float32, tag="ob", name=f"ob_{h}_{qh}"
                        )
                        if "ob_dve" in opts:
                            nc.vector.tensor_copy(ob_sb[:, :], ot[:, :])
                        else:
                            nc.scalar.copy(ob_sb[:, :], ot[:, :])
                        nc.sync.dma_start(
                            out=out[h][:, qh * QW:(qh + 1) * QW], in_=ob_sb[:, :]
                        )

            if repeats == 1:
                body(0)
            else:
                with tc.For_i(
                    0, repeats, 1,
                    hint_engines=(
                        mybir.EngineType.PE,
                        mybir.EngineType.DVE,
                        mybir.EngineType.Activation,
                        mybir.EngineType.Pool,
                    ),
                ):
                    body(0)

    nc.compile()
    return nc


def get_nc(repeats=1, qk_dtype=QK_DTYPE, opts=()):
    key = ("nc", repeats, qk_dtype, frozenset(opts))
    if key not in _CACHE:
        _CACHE[key] = _build(repeats, qk_dtype, opts)
    return _CACHE[key]


def prep_in_maps(q, k, v, mask, qk_dtype=QK_DTYPE, half="float16",
                 opts=DEFAULT_OPTS):
    avtt = "avtt" in opts
    sched, n_keep = _sched(opts)
    bf16 = np.float16 if half == "float16" else ml_dtypes.bfloat16
    qk_np = np.float32 if qk_dtype == "float32r" else bf16
    q = np.asarray(q, dtype=np.float32).reshape(B * H, S, D)
    k = np.asarray(k, dtype=np.float32).reshape(B * H, S, D)
    vv = np.asarray(v, dtype=np.float32).reshape(B * H, S, D)
    mask = np.asarray(mask).reshape(S, S)
    keepT = np.ascontiguousarray((1 - mask).T.astype(np.float32))  # [k, q]
    k4 = keepT.reshape(KB, P, QH, QW).transpose(2, 0, 1, 3)  # [QH, KB, P, QW]
    if "g3" in opts:
        # chunk-indexed keep layout matching the g3 QK emission order.
        keep4 = np.zeros((QH, 32, P, 512), dtype=np.float32)
        for qh in range(QH):
            for c in range(32):
                kbp, e, qc = c // 4, (c % 4) // 2, c % 2
                kb = 2 * kbp + e
                keep4[qh, c] = k4[qh, kb, :, qc * 512:(qc + 1) * 512]
        keep4 = np.ascontiguousarray(keep4).astype(bf16)
        eb4 = np.zeros((QH, N_EB, P, QW), dtype=bf16)
        in_maps = []
        for c in range(N_CORES):
            sl = slice(c * HPC, (c + 1) * HPC)
            in_maps.append({
                "qT": np.ascontiguousarray(
                    q[sl].transpose(0, 2, 1)).astype(qk_np),
                "kT": np.ascontiguousarray(
                    k[sl].transpose(0, 2, 1)).astype(qk_np),
                "v": vv[sl].astype(bf16),
                "vB": (vv[sl] * np.sqrt(2.0)).astype(bf16),
                "keepT": keep4,
                "ebT": eb4,
            })
        return in_maps
    # compacted keep (A pairs) and ebmask (S1/S2 pairs), in pair order.
    keep4 = np.zeros((QH, n_keep, P, QW), dtype=np.float32)
    eb4 = np.zeros((QH, N_EB, P, QW), dtype=np.float32)
    eb_s2 = EB_S2 if avtt else EB_S2PE
    for qh in range(QH):
        ai = 0
        si = 0
        for kbp in range(KB // 2):
            pth = sched[qh][kbp]
            blk = k4[qh, 2 * kbp:2 * kbp + 2]
            if pth == "A":
                keep4[qh, 2 * ai:2 * ai + 2] = blk
                ai += 1
            else:
                ebv = EB_S1 if pth == "S1" else eb_s2
                eb4[qh, 2 * si:2 * si + 2] = ebv + EB_MASKED * (1.0 - blk)
                si += 1
    keep4 = np.ascontiguousarray(keep4).astype(bf16)
    eb4 = np.ascontiguousarray(eb4).astype(bf16)
    in_maps = []
    for c in range(N_CORES):
        sl = slice(c * HPC, (c + 1) * HPC)
        in_maps.append({
            "qT": np.ascontiguousarray(q[sl].transpose(0, 2, 1)).astype(qk_np),
            "kT": np.ascontiguousarray(k[sl].transpose(0, 2, 1)).astype(qk_np),
            "v": vv[sl].astype(bf16),
            "vB": (vv[sl] * np.sqrt(2.0)).astype(bf16),
            "keepT": keep4,
            "ebT": eb4,
        })
    return in_maps


def finish_output(core_results):
    """core_results: list of [HPC, D+1, S] fp32 arrays -> [B, H, S, D] fp32."""
    outs = []
    for r in core_results:
        r = np.asarray(r, dtype=np.float32)
        o = (r[:, :D, :] / r[:, D:D + 1, :]).transpose(0, 2, 1)
        outs.append(o)
    return np.concatenate(outs, axis=0).reshape(B, H, S, D).astype(np.float32)


def kernel(q, k, v, mask):
    from concourse import bass_utils

    nc = get_nc(1, opts=DEFAULT_OPTS)
    in_maps = prep_in_maps(q, k, v, mask, opts=DEFAULT_OPTS)
    bkr = bass_utils.run_bass_kernel_spmd(nc, in_maps, list(range(N_CORES)))
    return finish_output([bkr.results[c]["out"] for c in range(N_CORES)])
